# revision 17
# baseline (speedup 1.0000x reference)
"""YOLO box-decode kernel for Trainium2 (Bass/Tile), 8-core data parallel.

Contract: kernel(**inputs) takes the FULL inputs from setup_inputs()
(x: [32,255,80,80] f32, anchors: [3,2] f32) and returns the full
(boxes [32,3,80,80,6] f32, mask [32,3,80,80] bool) like the reference.

Sharding: pure data parallel over the batch axis - 4 images per core,
no cross-core communication.

Per-core layout: 4 images x 3 anchors = 12 blocks, each a contiguous
[85, 6400] f32 region of DRAM. Three SBUF tiles of [128, 85, 200]: each
tile packs 4 blocks (partition groups of 32), so every DMA row is 200
contiguous f32 (800B) - above the 512B threshold where the DMA engines
run at full rate.

Compute per tile:
  ACT : sigmoid(ch0:2) + grid offsets, exp(ch2:4) * anchor dims
  DVE : score = cls * obj (in-place over cls, obj broadcast along ch)
        best  = reduce_max(score over ch)
        eq    = (score == best)            (u8)
        cand  = eq * (ch_index - 128)      (in-place over score)
        cls   = reduce_min(cand over ch) + 128
The eq/min construction reproduces jnp.argmax's first-occurrence
tie-breaking exactly; score/best/cls are bit-exact vs the f32 reference.
mask = best > 0.5 is derived on the host from best_score.
"""

import sys

for _p in ("/opt/trn_rl_repo", "/opt/pypackages"):
    if _p not in sys.path:
        sys.path.insert(0, _p)

import numpy as np

N, C, H, W = 32, 255, 80, 80
A = 3                 # anchors
V = 85                # values per anchor (5 + CLS)
CLS = 80
HW = H * W            # 6400
NCORES = 8
NSH = N // NCORES     # images per core
NBLK = NSH * A        # (n, a) blocks per core
TPB = 4               # blocks packed per SBUF tile (32 partitions each)
NT = NBLK // TPB      # SBUF tiles per core
J = HW // 32          # free-dim cells per partition (200)
BIG = 128.0           # argmax bias; any value > CLS with exact f32 ints
CONF_THR = 0.5


def _build(anchors: np.ndarray):
    import concourse.bass as bass
    import concourse.bacc as bacc
    import concourse.tile as tile
    from concourse import mybir

    f32 = mybir.dt.float32

    # Bacc (not plain Bass): its finalize() runs generate_event_semaphores,
    # which splits multi-semaphore waits - TRN2 allows 1 wait per instruction.
    nc = bacc.Bacc()
    x_h = nc.dram_tensor("x", [NSH, C, H, W], f32, kind="ExternalInput")
    out_h = nc.dram_tensor("out", [NSH, A, 6, HW], f32, kind="ExternalOutput")

    # Constant table, one row per partition:
    #   [0:200)   gx  grid x-offset per (partition, j) cell
    #   [200:400) gy  grid y-offset
    #   [400:403) anchor width for tile t (depends on p//32 -> block -> a)
    #   [403:406) anchor height for tile t
    #   [406:486) ch_index - BIG  (argmax payload)
    p32 = np.arange(128) % 32
    cell = p32[:, None] * J + np.arange(J)[None, :]
    gx = (cell % W).astype(np.float32)
    gy = (cell // W).astype(np.float32)
    anchw = np.zeros((128, NT), np.float32)
    anchh = np.zeros((128, NT), np.float32)
    for t in range(NT):
        for g in range(TPB):
            a = (t * TPB + g) % A
            anchw[g * 32:(g + 1) * 32, t] = anchors[a, 0]
            anchh[g * 32:(g + 1) * 32, t] = anchors[a, 1]
    iota = np.tile((np.arange(CLS) - BIG).astype(np.float32), (128, 1))
    cgrid_np = np.concatenate([gx, gy, anchw, anchh, iota], axis=1)
    cg_h = nc.inline_tensor(cgrid_np.astype(np.float32), name="cgrid")

    # whole shard as [NBLK*85, 6400]: 255 = 3*85, so the (n, a) blocks tile
    # the channel axis contiguously
    xv = x_h[:, :, :, :].rearrange("n c h w -> (n c) (h w)")
    ov = out_h[:, :, :, :].rearrange("n a k hw -> (n a k) hw")

    Sigmoid = mybir.ActivationFunctionType.Sigmoid
    Exp = mybir.ActivationFunctionType.Exp
    X = mybir.AxisListType.X
    op = mybir.AluOpType

    # j-split of the max-tree first step between GPSIMD (slower) and DVE,
    # chosen to balance measured rates (DVE ~1.05 ns/elem, pool ~2.2 ns/elem)
    JT1 = 138
    bf16 = mybir.dt.bfloat16

    with tile.TileContext(nc) as tc:
        with (
            tc.tile_pool(name="xp", bufs=2) as xp,
            tc.tile_pool(name="scrp", bufs=1) as scrp,
            tc.tile_pool(name="outp", bufs=2) as outp,
            tc.tile_pool(name="constp", bufs=1) as constp,
        ):
            cg = constp.tile([128, cgrid_np.shape[1]], f32)
            nc.gpsimd.dma_start(out=cg, in_=cg_h[:, :])
            gxy = cg[:, 0:400].rearrange("p (c j) -> p c j", c=2)
            iotb = constp.tile([128, CLS], bf16)
            nc.vector.tensor_copy(out=iotb, in_=cg[:, 406:406 + CLS])

            for t in range(NT):
                xt = xp.tile([128, V, J], f32)
                bx6 = outp.tile([128, 6, J], f32)
                # scr has two lives per tile: f32 scratch for the max tree,
                # then (overwritten) the bf16 eq/cand tensor in j-major layout
                scr = scrp.tile([128, J, CLS], bf16)
                scrf = (
                    scr[:, :, :]
                    .bitcast(f32)
                    .rearrange("p a b -> p (a b)")
                    .rearrange("p (c j) -> p c j", j=J)
                )  # [128, 40, J] f32 view of the same bytes

                for g in range(TPB):
                    b = t * TPB + g
                    ps = slice(g * 32, (g + 1) * 32)
                    src = xv[b * V:(b + 1) * V, :].rearrange(
                        "c (p j) -> p c j", j=J
                    )
                    nc.sync.dma_start(out=xt[ps, :, :], in_=src)
                    # first consumers are split per 32-partition group so each
                    # instruction waits on a single DMA semaphore (HW wait-slot
                    # limit).
                    nc.scalar.activation(
                        out=bx6[ps, 0:2, :], in_=xt[ps, 0:2, :], func=Sigmoid
                    )
                    nc.scalar.activation(
                        out=bx6[ps, 2:4, :], in_=xt[ps, 2:4, :], func=Exp
                    )
                    # score = cls * obj, in place over the cls channels (pool)
                    nc.gpsimd.tensor_tensor(
                        out=xt[ps, 5:V, :],
                        in0=xt[ps, 5:V, :],
                        in1=xt[ps, 4:5, :].broadcast_to((32, CLS, J)),
                        op=op.mult,
                    )

                # centers: sigmoid(tx, ty) + grid
                nc.vector.tensor_add(out=bx6[:, 0:2, :], in0=bx6[:, 0:2, :], in1=gxy)
                # sizes: exp(tw, th) * anchor (ACT: copy with per-partition scale)
                nc.scalar.mul(
                    out=bx6[:, 2, :], in_=bx6[:, 2, :], mul=cg[:, 400 + t:401 + t]
                )
                nc.scalar.mul(
                    out=bx6[:, 3, :], in_=bx6[:, 3, :], mul=cg[:, 403 + t:404 + t]
                )

                # best = max over ch via pairwise max tree: every step has a
                # unit-stride inner dim (strided tensor_reduce measured 1.7x
                # slower than this). All DVE: the pool ucode only supports
                # basic arithmetic TensorTensor ops (max/is_equal rejected).
                nc.vector.tensor_tensor(
                    out=scrf[:, 0:40, :],
                    in0=xt[:, 5:45, :],
                    in1=xt[:, 45:85, :],
                    op=op.max,
                )
                for w in (20, 10, 5):
                    nc.vector.tensor_tensor(
                        out=scrf[:, 0:w, :], in0=scrf[:, 0:w, :],
                        in1=scrf[:, w:2 * w, :], op=op.max,
                    )
                nc.vector.tensor_tensor(
                    out=scrf[:, 0:2, :], in0=scrf[:, 0:2, :], in1=scrf[:, 2:4, :],
                    op=op.max,
                )
                nc.vector.tensor_tensor(
                    out=scrf[:, 0, :], in0=scrf[:, 0, :], in1=scrf[:, 1, :],
                    op=op.max,
                )
                nc.vector.tensor_tensor(
                    out=bx6[:, 4, :], in0=scrf[:, 0, :], in1=scrf[:, 4, :],
                    op=op.max,
                )

                # eq = (score == best) -> bf16 j-major (DVE only: the pool
                # ucode has no comparison ops)
                nc.vector.tensor_tensor(
                    out=scr,
                    in0=xt[:, 5:V, :].transpose([0, 2, 1]),
                    in1=bx6[:, 4, :].unsqueeze(2).broadcast_to((128, J, CLS)),
                    op=op.is_equal,
                )
                # cand = eq * (ch - BIG): all-bf16 unit stride (DVE 2x mode)
                nc.vector.tensor_tensor(
                    out=scr,
                    in0=scr,
                    in1=iotb.unsqueeze(1).broadcast_to((128, J, CLS)),
                    op=op.mult,
                )
                # best_cls = min(cand) + BIG; contiguous bf16 reduce
                nc.vector.tensor_reduce(
                    out=bx6[:, 5, :], in_=scr, axis=X, op=op.min
                )
                nc.vector.tensor_scalar_add(
                    out=bx6[:, 5, :], in0=bx6[:, 5, :], scalar1=BIG
                )

                for g in range(TPB):
                    b = t * TPB + g
                    dst = ov[b * 6:(b + 1) * 6, :].rearrange(
                        "k (p j) -> p k j", j=J
                    )
                    # SWDGE (single queue/semaphore) so downstream WAR waits on
                    # bx6 coalesce into one wait slot
                    nc.gpsimd.dma_start(
                        out=dst, in_=bx6[g * 32:(g + 1) * 32, :, :]
                    )

    return nc


def _assemble(core_outs):
    out = np.concatenate(core_outs, axis=0)           # [N, A, 6, HW]
    nb = out.shape[0]
    boxes = np.transpose(out, (0, 1, 3, 2)).reshape(nb, A, H, W, 6)
    boxes = np.ascontiguousarray(boxes, dtype=np.float32)
    mask = boxes[..., 4] > CONF_THR
    return boxes, mask


def kernel(**inputs):
    x = np.ascontiguousarray(inputs["x"], dtype=np.float32)
    anchors = np.asarray(inputs["anchors"], dtype=np.float32)
    assert x.shape == (N, C, H, W), x.shape

    from concourse.bass_utils import run_bass_kernel_spmd

    nc = _build(anchors)
    nc.finalize()  # Bacc lowering (reg alloc, wait splitting) before PJRT
    in_maps = [{"x": x[k * NSH:(k + 1) * NSH]} for k in range(NCORES)]
    res = run_bass_kernel_spmd(nc, in_maps, list(range(NCORES))).results
    return _assemble([res[k]["out"] for k in range(NCORES)])


# revision 20
# speedup vs baseline: 1.8642x; 1.8642x over previous
"""YOLO box-decode kernel for Trainium2 (Bass/Tile), 8-core data parallel.

Contract: kernel(**inputs) takes the FULL inputs from setup_inputs()
(x: [32,255,80,80] f32, anchors: [3,2] f32) and returns the full
(boxes [32,3,80,80,6] f32, mask [32,3,80,80] bool) like the reference.

Sharding: pure data parallel over the batch axis - 4 images per core,
no cross-core communication.

Per-core layout: 4 images x 3 anchors = 12 blocks, each a contiguous
[85, 6400] f32 region of DRAM. Three SBUF tiles of [128, 85, 200]: each
tile packs 4 blocks (partition groups of 32), so every DMA row is 200
contiguous f32 (800B) - above the 512B threshold where the DMA engines
run at full rate.

Compute per tile:
  ACT : sigmoid(ch0:2) + grid offsets, exp(ch2:4) * anchor dims
  DVE : score = cls * obj (in-place over cls, obj broadcast along ch)
        best  = reduce_max(score over ch)
        eq    = (score == best)            (u8)
        cand  = eq * (ch_index - 128)      (in-place over score)
        cls   = reduce_min(cand over ch) + 128
The eq/min construction reproduces jnp.argmax's first-occurrence
tie-breaking exactly; score/best/cls are bit-exact vs the f32 reference.
mask = best > 0.5 is derived on the host from best_score.
"""

import sys

for _p in ("/opt/trn_rl_repo", "/opt/pypackages"):
    if _p not in sys.path:
        sys.path.insert(0, _p)

import numpy as np

N, C, H, W = 32, 255, 80, 80
A = 3                 # anchors
V = 85                # values per anchor (5 + CLS)
CLS = 80
HW = H * W            # 6400
NCORES = 8
NSH = N // NCORES     # images per core
NBLK = NSH * A        # (n, a) blocks per core
TPB = 4               # blocks packed per SBUF tile (32 partitions each)
NT = NBLK // TPB      # SBUF tiles per core
J = HW // 32          # free-dim cells per partition (200)
BIG = 128.0           # argmax bias; any value > CLS with exact f32 ints
CONF_THR = 0.5


def _build(anchors: np.ndarray):
    import concourse.bass as bass
    import concourse.bacc as bacc
    import concourse.tile as tile
    from concourse import mybir

    f32 = mybir.dt.float32

    # Bacc (not plain Bass): its finalize() runs generate_event_semaphores,
    # which splits multi-semaphore waits - TRN2 allows 1 wait per instruction.
    nc = bacc.Bacc()
    x_h = nc.dram_tensor("x", [NSH, C, H, W], f32, kind="ExternalInput")
    out_h = nc.dram_tensor("out", [NSH, A, 6, HW], f32, kind="ExternalOutput")

    # Constant table, one row per partition:
    #   [0:200)   gx  grid x-offset per (partition, j) cell
    #   [200:400) gy  grid y-offset
    #   [400:403) anchor width for tile t (depends on p//32 -> block -> a)
    #   [403:406) anchor height for tile t
    #   [406:486) ch_index - BIG  (argmax payload)
    p32 = np.arange(128) % 32
    cell = p32[:, None] * J + np.arange(J)[None, :]
    gx = (cell % W).astype(np.float32)
    gy = (cell // W).astype(np.float32)
    anchw = np.zeros((128, NT), np.float32)
    anchh = np.zeros((128, NT), np.float32)
    for t in range(NT):
        for g in range(TPB):
            a = (t * TPB + g) % A
            anchw[g * 32:(g + 1) * 32, t] = anchors[a, 0]
            anchh[g * 32:(g + 1) * 32, t] = anchors[a, 1]
    iota = np.tile((np.arange(CLS) - BIG).astype(np.float32), (128, 1))
    cgrid_np = np.concatenate([gx, gy, anchw, anchh, iota], axis=1)
    cg_h = nc.inline_tensor(cgrid_np.astype(np.float32), name="cgrid")

    # whole shard as [NBLK*85, 6400]: 255 = 3*85, so the (n, a) blocks tile
    # the channel axis contiguously
    xv = x_h[:, :, :, :].rearrange("n c h w -> (n c) (h w)")
    ov = out_h[:, :, :, :].rearrange("n a k hw -> (n a k) hw")

    Sigmoid = mybir.ActivationFunctionType.Sigmoid
    Exp = mybir.ActivationFunctionType.Exp
    X = mybir.AxisListType.X
    op = mybir.AluOpType

    bf16 = mybir.dt.bfloat16
    # j-split of the max-tree first step between GPSIMD and DVE, balancing
    # measured rates (DVE ~0.53 ns/elem unit-stride pair-max vs pool ~2.2)
    JT1 = 100

    with tile.TileContext(nc) as tc:
        with (
            tc.tile_pool(name="xsp", bufs=1) as xsp,
            tc.tile_pool(name="xcp", bufs=1) as xcp,
            tc.tile_pool(name="sjp", bufs=1) as sjp,
            tc.tile_pool(name="scrp", bufs=1) as scrp,
            tc.tile_pool(name="outp", bufs=2) as outp,
            tc.tile_pool(name="constp", bufs=1) as constp,
        ):
            cg = constp.tile([128, cgrid_np.shape[1]], f32)
            nc.gpsimd.dma_start(out=cg, in_=cg_h[:, :])
            gxy = cg[:, 0:400].rearrange("p (c j) -> p c j", c=2)
            iotb = constp.tile([128, CLS], bf16)
            nc.vector.tensor_copy(out=iotb, in_=cg[:, 406:406 + CLS])

            for t in range(NT):
                # xs: xy/wh/obj channels; xc: class channels (dead after the
                # score multiply); sj: score in j-major layout so every DVE
                # pass downstream is unit-stride
                xs = xsp.tile([128, 5, J], f32)
                xc = xcp.tile([128, CLS, J], f32)
                sj = sjp.tile([128, J, CLS], f32)
                bx6 = outp.tile([128, 6, J], f32)
                # scr: f32 max-tree scratch, then (bitcast) bf16 eq/cand
                scr = scrp.tile([128, J, 40], f32)
                scrb = (
                    scr[:, :, :]
                    .rearrange("p a b -> p (a b)")
                    .bitcast(bf16)
                    .rearrange("p (j c) -> p j c", c=CLS)
                )  # [128, J, CLS] bf16 view of the same bytes

                for g in range(TPB):
                    b = t * TPB + g
                    ps = slice(g * 32, (g + 1) * 32)
                    nc.sync.dma_start(
                        out=xs[ps, :, :],
                        in_=xv[b * V:b * V + 5, :].rearrange(
                            "c (p j) -> p c j", j=J
                        ),
                    )
                    nc.sync.dma_start(
                        out=xc[ps, :, :],
                        in_=xv[b * V + 5:(b + 1) * V, :].rearrange(
                            "c (p j) -> p c j", j=J
                        ),
                    )
                # score = cls * obj on pool, written j-major (the pool's
                # software address generation is stride-insensitive)
                nc.gpsimd.tensor_tensor(
                    out=sj[:, :, :].transpose([0, 2, 1]),
                    in0=xc,
                    in1=xs[:, 4:5, :].broadcast_to((128, CLS, J)),
                    op=op.mult,
                )

                # centers/sizes on ACT (batched by function to avoid
                # activation-table reloads), grid add on DVE
                nc.scalar.activation(
                    out=bx6[:, 0:2, :], in_=xs[:, 0:2, :], func=Sigmoid
                )
                nc.scalar.activation(
                    out=bx6[:, 2:4, :], in_=xs[:, 2:4, :], func=Exp
                )
                nc.vector.tensor_add(out=bx6[:, 0:2, :], in0=bx6[:, 0:2, :], in1=gxy)
                nc.scalar.mul(
                    out=bx6[:, 2, :], in_=bx6[:, 2, :], mul=cg[:, 400 + t:401 + t]
                )
                nc.scalar.mul(
                    out=bx6[:, 3, :], in_=bx6[:, 3, :], mul=cg[:, 403 + t:404 + t]
                )

                # best = max over ch: pairwise max tree, all unit-stride in
                # j-major layout. All on DVE - the pool ucode rejects max.
                nc.vector.tensor_tensor(
                    out=scr,
                    in0=sj[:, :, 0:40],
                    in1=sj[:, :, 40:80],
                    op=op.max,
                )
                for w in (20, 10, 5):
                    nc.vector.tensor_tensor(
                        out=scr[:, :, 0:w], in0=scr[:, :, 0:w],
                        in1=scr[:, :, w:2 * w], op=op.max,
                    )
                nc.vector.tensor_tensor(
                    out=scr[:, :, 0:2], in0=scr[:, :, 0:2], in1=scr[:, :, 2:4],
                    op=op.max,
                )
                nc.vector.tensor_tensor(
                    out=scr[:, :, 0], in0=scr[:, :, 0], in1=scr[:, :, 1],
                    op=op.max,
                )
                nc.vector.tensor_tensor(
                    out=bx6[:, 4, :], in0=scr[:, :, 0], in1=scr[:, :, 4],
                    op=op.max,
                )

                # eq = (score == best) -> bf16, everything unit-stride
                nc.vector.tensor_tensor(
                    out=scrb,
                    in0=sj,
                    in1=bx6[:, 4, :].unsqueeze(2).broadcast_to((128, J, CLS)),
                    op=op.is_equal,
                )
                # cand = eq * (ch - BIG): all-bf16 unit stride (DVE 2x mode)
                nc.vector.tensor_tensor(
                    out=scrb,
                    in0=scrb,
                    in1=iotb.unsqueeze(1).broadcast_to((128, J, CLS)),
                    op=op.mult,
                )
                # best_cls = min(cand) + BIG; reduce kept all-bf16 to give
                # the DVE 2x perf mode a chance, f32 convert in the add
                cmin = outp.tile([128, J], bf16, tag="cmin")
                nc.vector.tensor_reduce(
                    out=cmin, in_=scrb, axis=X, op=op.min
                )
                nc.vector.tensor_scalar_add(
                    out=bx6[:, 5, :], in0=cmin, scalar1=BIG
                )

                for g in range(TPB):
                    b = t * TPB + g
                    dst = ov[b * 6:(b + 1) * 6, :].rearrange(
                        "k (p j) -> p k j", j=J
                    )
                    # SWDGE (single queue/semaphore) so downstream WAR waits on
                    # bx6 coalesce into one wait slot
                    nc.gpsimd.dma_start(
                        out=dst, in_=bx6[g * 32:(g + 1) * 32, :, :]
                    )

    return nc


def _assemble(core_outs):
    out = np.concatenate(core_outs, axis=0)           # [N, A, 6, HW]
    nb = out.shape[0]
    boxes = np.transpose(out, (0, 1, 3, 2)).reshape(nb, A, H, W, 6)
    boxes = np.ascontiguousarray(boxes, dtype=np.float32)
    mask = boxes[..., 4] > CONF_THR
    return boxes, mask


def kernel(**inputs):
    x = np.ascontiguousarray(inputs["x"], dtype=np.float32)
    anchors = np.asarray(inputs["anchors"], dtype=np.float32)
    assert x.shape == (N, C, H, W), x.shape

    from concourse.bass_utils import run_bass_kernel_spmd

    nc = _build(anchors)
    nc.finalize()  # Bacc lowering (reg alloc, wait splitting) before PJRT
    in_maps = [{"x": x[k * NSH:(k + 1) * NSH]} for k in range(NCORES)]
    res = run_bass_kernel_spmd(nc, in_maps, list(range(NCORES))).results
    return _assemble([res[k]["out"] for k in range(NCORES)])
